# revision 17
# baseline (speedup 1.0000x reference)
"""Bass/Tile TRN2 kernel for nn_DecomposedRotateNet (dense_mlp).

Contract: kernel(**inputs) takes FULL unsharded numpy inputs (as produced by
setup_inputs()) and returns the FULL [4096, 64] float32 output.

Strategy: pure data parallel over 8 NeuronCores -- batch 4096 -> 512 rows/core,
small MLP weights replicated; all layout prep on host.

v2: the dominant index-net matmuls run in fp8 e4m3 with DoubleRow perf mode
(2 fp8 weights/PE cell -> K=256 per matmul), with power-of-two scales folded
into the weights/biases host-side so intermediate activations sit in fp8's
good range.  The per-position softmax column sums are accumulated directly
into two PSUM banks via one-hot-column stationary operands (out row t <-
colsum of position t), so the normalize is two [64,512] ops at the very end
instead of per-position work.  Elementwise work is spread across DVE, ACT
and GPSIMD.
"""

import os
import sys
import functools

import numpy as np

for _p in ("/opt/trn_rl_repo",):
    if _p not in sys.path and os.path.isdir(_p):
        sys.path.insert(0, _p)

import concourse.bacc as bacc
import concourse.bass as bass
import concourse.mybir as mybir
import concourse.tile as tile
from concourse import bass_utils
from concourse._compat import with_exitstack
from contextlib import ExitStack

B, BITS, HID = 4096, 64, 512
NCORES = 8
BC = B // NCORES          # 512 batch rows per core
NBT = BC // 128           # 4 batch tiles of 128 (phase 1)
NKC = HID // 128          # 4 chunks of the hidden dim
EPS = 1e-5

# fp8 scale folding for the index-net (phase 2) matmuls
S1 = 64.0                 # h1 activation scale
S2 = 32.0                 # Wi2 weight scale
S3 = S1 * S2              # h2 activation scale (folded: no rescale op needed)
S4 = 64.0                 # Wi3 weight scale

F32 = mybir.dt.float32
BF16 = mybir.dt.bfloat16
FP8 = mybir.dt.float8e4

DR = mybir.MatmulPerfMode.DoubleRow


@with_exitstack
def _build_kernel(ctx: ExitStack, tc: "tile.TileContext", io: dict, affine: bool):
    nc = tc.nc
    AF = mybir.ActivationFunctionType
    ALU = mybir.AluOpType

    persist = ctx.enter_context(tc.tile_pool(name="persist", bufs=1))

    def load(name, shape, dt):
        t = persist.tile(shape, dt, name=f"sb_{name}", tag=f"sb_{name}")
        nc.sync.dma_start(t[:], io[name][:])
        return t

    def load_on(eng, name, shape, dt):
        t = persist.tile(shape, dt, name=f"sb_{name}", tag=f"sb_{name}")
        eng.dma_start(t[:], io[name][:])
        return t

    # ---- persistent SBUF tensors --------------------------------------
    # phase-1-critical tensors go first (and on separate queues) so the
    # first matmuls don't wait behind the phase-2 weight loads.
    w1t = load("w1t", [BITS + 1, HID], BF16)      # [W1.T ; b1]
    ident_early = None
    w2ta = load_on(nc.gpsimd, "w2ta", [128, NKC, HID], BF16)
    w2b = load_on(nc.gpsimd, "w2b", [1, HID], BF16)
    w3ta = load_on(nc.gpsimd, "w3ta", [128, NKC, BITS], BF16)
    w3b = load_on(nc.gpsimd, "w3b", [1, BITS], BF16)
    w2t = [w2ta[:, i, :] for i in range(NKC)]
    w3t = [w3ta[:, i, :] for i in range(NKC)]
    aTb2 = load_on(nc.scalar, "aTb2", [128, BC], BF16)  # a_bits .T, duplicated
    if affine:
        g1bc = load("g1bc", [128, HID], F32)
        be1bc = load("be1bc", [128, HID], F32)
        g2bc = load("g2bc", [128, HID], F32)
        be2bc = load("be2bc", [128, HID], F32)
    wi1bt = load_on(nc.gpsimd, "wi1bt", [BITS, HID], BF16)  # Wi1[:, 64:].T * S1
    posb = load_on(nc.scalar, "posb", [128, NKC * BITS], F32)
    w2dr = [load_on(nc.scalar, f"w2dr{p}", [128, 2, HID], FP8) for p in range(2)]
    bi2c = load_on(nc.scalar, "bi2c", [128, NKC], F32)
    w3dr = [load_on(nc.scalar, f"w3dr{p}", [128, 2, BITS], FP8) for p in range(2)]
    bi3c2 = load_on(nc.scalar, "bi3c2", [128, 1], F32)      # bi3 duplicated
    zsel2 = load_on(nc.scalar, "zsel2", [128, 191], BF16)   # pair one-hot columns

    # x0a = [shift_bits_T shard ; ones row] for the z1 matmul (bias fold)
    x0a = persist.tile([BITS + 1, BC], BF16, name="x0a", tag="x0a")
    nc.sync.dma_start(x0a[0:BITS, :], io["sbT"][:])
    nc.sync.dma_start(x0a[BITS : BITS + 1, :], io["onesr"][:])
    ones1r = persist.tile([1, BC], BF16, name="ones1r", tag="ones1r")
    nc.sync.dma_start(ones1r[:], io["onesr"][:])
    ident = load("ident", [128, 128], BF16)

    epsc = persist.tile([128, 1], F32, name="epsc", tag="epsc")
    nc.vector.memset(epsc[:], EPS)
    # prefetch activation-function tables while input DMAs run, so the
    # first LN/softmax ops don't serialize behind ACT_TABLE_LOADs
    warmup = persist.tile([1, 1], F32, name="warmup", tag="warmup")
    for fn in (AF.Sqrt, AF.Relu, AF.Exp, AF.Identity):
        nc.scalar.activation(warmup[:], epsc[0:1, :], fn)

    ssT = persist.tile([BITS, BC], BF16, name="ssT", tag="ssT")      # shift_soft.T
    shiftT = persist.tile([128, NKC, BC], BF16, name="shiftT", tag="shiftT")

    # =================== phase 1: shift decoder =======================
    # Stage-major across the 4 batch tiles so the serial LN dependency
    # chains of different tiles overlap on DVE/ACT while PE runs matmuls.
    with (
        tc.tile_pool(name="p1s", bufs=4) as p1s,
        tc.tile_pool(name="p1p", bufs=2, space="PSUM") as p1p,
    ):
        def layernorm_relu(z_psum, g, be, out_tag, bt):
            """z [128, 512] PSUM -> relu(LN(z)*g+be) [128, 512] SBUF bf16."""
            stats = p1s.tile([128, 6], F32, tag=f"stats{bt}", name="stats")
            nc.vector.bn_stats(stats[:], z_psum[:])
            mv = p1s.tile([128, 2], F32, tag=f"mv{bt}", name="mv")
            nc.vector.bn_aggr(mv[:], stats[:])
            std = p1s.tile([128, 1], F32, tag=f"std{bt}", name="std")
            nc.scalar.activation(std[:], mv[:, 1:2], AF.Sqrt, bias=epsc[:])
            rinv = p1s.tile([128, 1], F32, tag=f"rinv{bt}", name="rinv")
            nc.vector.reciprocal(rinv[:], std[:])
            nmr = p1s.tile([128, 1], F32, tag=f"nmr{bt}", name="nmr")
            # nmr = (mean * -1) * rinv
            nc.vector.scalar_tensor_tensor(
                nmr[:], mv[:, 0:1], -1.0, rinv[:], op0=ALU.mult, op1=ALU.mult
            )
            if not affine:
                # g==1, be==0: relu(LN(z)) in one ACT op
                a = p1s.tile([128, HID], BF16, tag=out_tag, name="a")
                nc.scalar.activation(a[:], z_psum[:], AF.Relu, bias=nmr[:], scale=rinv[:])
                return a
            xn = p1s.tile([128, HID], F32, tag=f"xn{bt}", name="xn")
            nc.scalar.activation(xn[:], z_psum[:], AF.Identity, bias=nmr[:], scale=rinv[:])
            t1 = p1s.tile([128, HID], F32, tag=f"t1{bt}", name="t1")
            nc.vector.tensor_tensor(t1[:], xn[:], g[:], op=ALU.mult)
            t2 = p1s.tile([128, HID], F32, tag=f"t2{bt}", name="t2")
            nc.vector.tensor_tensor(t2[:], t1[:], be[:], op=ALU.add)
            a = p1s.tile([128, HID], BF16, tag=out_tag, name="a")
            nc.vector.tensor_scalar_max(a[:], t2[:], 0.0)
            return a

        dma_engs = [nc.sync, nc.scalar]

        def transpose128(src, cols, out_tag):
            """src [128, cols*128] SBUF bf16 -> list of [128,128] transposed
            via the (otherwise idle) DMA xbar engines."""
            outs = []
            for h in range(cols):
                sb = p1s.tile([128, 128], BF16, tag=f"{out_tag}{h}", name="sb")
                dma_engs[h % 2].dma_start_transpose(
                    sb[:], src[:, h * 128 : (h + 1) * 128]
                )
                outs.append(sb)
            return outs

        g1 = g1bc if affine else None
        be1 = be1bc if affine else None
        g2 = g2bc if affine else None
        be2 = be2bc if affine else None
        bss = [slice(bt * 128, (bt + 1) * 128) for bt in range(NBT)]
        z1s, a1s, a1Ts, z2s, a2s, a2Ts, z3s = {}, {}, {}, {}, {}, {}, {}
        for bt in range(NBT):
            z1s[bt] = p1p.tile([128, HID], F32, tag=f"zz{bt}", name="z1", bufs=1)
            nc.tensor.matmul(z1s[bt][:], x0a[:, bss[bt]], w1t[:], start=True, stop=True)
        for bt in range(NBT):
            a1s[bt] = layernorm_relu(z1s[bt], g1, be1, f"a1_{bt}", bt)
        for bt in range(NBT):
            a1Ts[bt] = transpose128(a1s[bt], NKC, f"a1T_{bt}_")
            z2s[bt] = p1p.tile([128, HID], F32, tag=f"zz{bt}", name="z2", bufs=1)
            for h in range(NKC):
                nc.tensor.matmul(z2s[bt][:], a1Ts[bt][h][:], w2t[h], start=(h == 0), stop=(not affine and h == NKC - 1))
            if affine:
                nc.tensor.matmul(z2s[bt][:], ones1r[:, bss[bt]], w2b[:], start=False, stop=True)
        for bt in range(NBT):
            a2s[bt] = layernorm_relu(z2s[bt], g2, be2, f"a2_{bt}", bt)
        for bt in range(NBT):
            a2Ts[bt] = transpose128(a2s[bt], NKC, f"a2T_{bt}_")
            z3s[bt] = p1p.tile([128, BITS], F32, tag=f"z3_{bt%2}", name="z3", bufs=1)
            for h in range(NKC):
                nc.tensor.matmul(z3s[bt][:], a2Ts[bt][h][:], w3t[h], start=(h == 0), stop=(not affine and h == NKC - 1))
            if affine:
                nc.tensor.matmul(z3s[bt][:], ones1r[:, bss[bt]], w3b[:], start=False, stop=True)
        for bt in range(NBT):
            # softmax over free dim (64)
            z3 = z3s[bt]
            mx = p1s.tile([128, 1], F32, tag=f"mx{bt}", name="mx")
            nc.vector.reduce_max(mx[:], z3[:], axis=mybir.AxisListType.X)
            nmx = p1s.tile([128, 1], F32, tag=f"nmx{bt}", name="nmx")
            nc.vector.tensor_scalar_mul(nmx[:], mx[:], -1.0)
            es = p1s.tile([128, BITS], F32, tag=f"es{bt}", name="es")
            ssum = p1s.tile([128, 1], F32, tag=f"ssum{bt}", name="ssum")
            nc.scalar.activation(es[:], z3[:], AF.Exp, bias=nmx[:], accum_out=ssum[:])
            rs = p1s.tile([128, 1], F32, tag=f"rs{bt}", name="rs")
            nc.vector.reciprocal(rs[:], ssum[:])
            ss = p1s.tile([128, BITS], BF16, tag=f"ss{bt}", name="ss")
            nc.vector.tensor_scalar_mul(ss[:], es[:], rs[:])
            tps = p1p.tile([BITS, 128], BF16, tag="tp", bufs=2, name="tps")
            nc.tensor.transpose(tps[:], ss[:], ident[:])
            nc.vector.tensor_copy(ssT[:, bss[bt]], tps[:])

        # ---- phase 1.5: shiftT = (Wi1[:,64:]*S1).T-chunks @ shift_soft.T
        sps = {}
        for fc in range(NKC):
            sps[fc] = p1p.tile([128, BC], F32, tag=f"zz{fc}", name="sp", bufs=1)
            nc.tensor.matmul(
                sps[fc][:], wi1bt[:, fc * 128 : (fc + 1) * 128], ssT[:], start=True, stop=True
            )
        for fc in range(NKC):
            nc.vector.tensor_copy(shiftT[:, fc, :], sps[fc][:])

    # =================== phase 2: index net (fp8 DoubleRow) ============
    # 64 output positions t, processed in 16 blocks of 4 (BLK) so each
    # MM1 weight tile is loaded once per block (LDWEIGHTS amortized 4x).
    # Positions are paired for the epilogue: MM2 writes logits for even t
    # to PSUM partitions 0-63 and odd t to 64-127, so exp / a-weighting /
    # column sums run on full [128, 512] tiles (one op per 2 positions).
    # Column sums accumulate into sAll/dAll PSUM banks via one-hot-column
    # stationary operands; the final normalize is 2 ops + 1 DMA.
    NP = BITS  # 64 output positions
    BLK = 4
    NBLK = NP // BLK
    with (
        tc.tile_pool(name="p2s", bufs=2) as p2s,
        tc.tile_pool(name="p2z", bufs=1, space="PSUM") as p2z,
        tc.tile_pool(name="p2lg", bufs=1, space="PSUM") as p2lg,
        tc.tile_pool(name="p2acc", bufs=1, space="PSUM") as p2acc,
    ):
        sAll = p2acc.tile([BITS, BC], F32, tag="sAll")
        dAll = p2acc.tile([BITS, BC], F32, tag="dAll")

        st = {}

        def h1_ops(g, slot):
            """h1(t) = relu(shiftT + posb[:, t-col]) -> fp8, 4 chunks."""
            t = g * BLK + slot
            h1 = st[(g, "h1", slot)]
            for fc in range(NKC):
                b = posb[:, fc * BITS + t : fc * BITS + t + 1]
                dst = h1[:, fc, :]
                src = shiftT[:, fc, :]
                if fc % 2 == 0:
                    nc.vector.tensor_scalar(
                        dst, src, b, 0.0, op0=ALU.add, op1=ALU.max
                    )
                else:
                    nc.scalar.activation(dst, src, AF.Relu, bias=b)

        def alloc_h1_block(g):
            for slot in range(BLK):
                st[(g, "h1", slot)] = p2s.tile(
                    [128, NKC, BC], FP8, tag=f"h1_{slot}", name=f"h1_{slot}"
                )

        def expcs_pair(g, pair):
            """exp + a-weight + column sums for pair (2 positions)."""
            lgp = st[(g, "lg", pair)]
            e2 = p2s.tile([128, BC], BF16, tag="e2", name="e2")
            nc.scalar.activation(
                e2[:], lgp[:], AF.Exp, bias=bi3c2[:], scale=1.0 / (S3 * S4)
            )
            tmp2 = p2s.tile([128, BC], BF16, tag="tmp2", name="tmp2")
            nc.vector.tensor_tensor(tmp2[:], e2[:], aTb2[:], op=ALU.mult)
            pi = g * 2 + pair                     # global pair index [0, 32)
            sel = zsel2[:, 127 - 2 * pi : 191 - 2 * pi]
            nc.tensor.matmul(
                sAll[:], sel, e2[:],
                start=(pi == 0), stop=(pi == NP // 2 - 1), skip_group_check=True,
            )
            nc.tensor.matmul(
                dAll[:], sel, tmp2[:],
                start=(pi == 0), stop=(pi == NP // 2 - 1), skip_group_check=True,
            )
            del st[(g, "lg", pair)]

        def mm1_block(g):
            """MM1 for 4 positions, weight-stationary over (p, kc); h2 ops
            drain each kc round; prev block's epilogue and next block's h1
            are interleaved to keep all engines fed."""
            h2b = [
                p2s.tile([128, NKC, BC], FP8, tag=f"h2_{s}", name=f"h2_{s}")
                for s in range(BLK)
            ]
            for kc in range(NKC):
                z = [
                    p2z.tile([128, BC], F32, tag=f"z{s}", name=f"z{s}")
                    for s in range(BLK)
                ]
                for p in range(2):
                    w = w2dr[p][:, :, kc * 128 : (kc + 1) * 128]
                    for s in range(BLK):
                        nc.tensor.matmul(
                            z[s][:],
                            w,
                            st[(g, "h1", s)][:, 2 * p : 2 * p + 2, :],
                            start=(p == 0),
                            stop=(p == 1),
                            perf_mode=DR,
                            skip_group_check=True,
                        )
                # per-slot h2 so each z bank frees as early as possible
                b = bi2c[:, kc : kc + 1]
                for s in range(BLK):
                    dst = h2b[s][:, kc, :]
                    if (kc + s) % 2 == 0:
                        nc.vector.tensor_scalar(
                            dst, z[s][:], b, 0.0, op0=ALU.add, op1=ALU.max
                        )
                    else:
                        nc.scalar.activation(dst, z[s][:], AF.Relu, bias=b)
                # interleave: prev-block mm2 + epilogue pairs, next-block h1
                # (h1 production front-loaded into the PE-heavy kc0 round)
                if kc == 0 and g >= 1:
                    mm2_block(g - 1)
                elif kc == 1 and g >= 1:
                    expcs_pair(g - 1, 0)
                elif kc == 2 and g >= 1:
                    expcs_pair(g - 1, 1)
                if g + 1 < NBLK:
                    if kc == 0:
                        h1_ops(g + 1, 0)
                        h1_ops(g + 1, 1)
                    elif kc == 1:
                        h1_ops(g + 1, 2)
                    elif kc == 2:
                        h1_ops(g + 1, 3)
            for s in range(BLK):
                st[(g, "h2", s)] = h2b[s]
                del st[(g, "h1", s)]

        def mm2_block(g):
            for pair in range(2):
                lgp = p2lg.tile([128, BC], F32, tag=f"lgp{pair}", name=f"lgp{pair}")
                for par in range(2):
                    s = 2 * pair + par
                    h2 = st[(g, "h2", s)]
                    out = lgp[64 * par : 64 * (par + 1), :]
                    if par == 0:
                        # DoubleRow requires dst partition 0
                        for p in range(2):
                            nc.tensor.matmul(
                                out,
                                w3dr[p][:, :, :],
                                h2[:, 2 * p : 2 * p + 2, :],
                                start=(p == 0),
                                stop=(p == 1),
                                perf_mode=DR,
                                skip_group_check=True,
                            )
                    else:
                        # odd slot -> partitions 64-127: plain fp8 matmuls
                        for fc in range(NKC):
                            nc.tensor.matmul(
                                out,
                                w3dr[fc // 2][:, fc % 2, :],
                                h2[:, fc, :],
                                start=(fc == 0),
                                stop=(fc == NKC - 1),
                                skip_group_check=True,
                            )
                st[(g, "lg", pair)] = lgp
            for s in range(BLK):
                del st[(g, "h2", s)]

        alloc_h1_block(0)
        for slot in range(BLK):
            h1_ops(0, slot)
        for g in range(NBLK):
            if g + 1 < NBLK:
                alloc_h1_block(g + 1)
            mm1_block(g)
        mm2_block(NBLK - 1)
        expcs_pair(NBLK - 1, 0)
        expcs_pair(NBLK - 1, 1)

        # final normalize: out[t, b] = dAll/sAll
        r = p2s.tile([BITS, BC], F32, tag="r")
        nc.vector.reciprocal(r[:], sAll[:])
        outT = p2s.tile([BITS, BC], F32, tag="outT")
        nc.vector.tensor_tensor(outT[:], dAll[:], r[:], op=ALU.mult)
        nc.sync.dma_start(io["out_t"][:], outT[:])


def _input_specs(affine: bool):
    specs = [
        ("sbT", [BITS, BC], BF16),
        ("aTb2", [128, BC], BF16),
        ("onesr", [1, BC], BF16),
        ("w1t", [BITS + 1, HID], BF16),
        ("w2ta", [128, NKC, HID], BF16),
        ("w2b", [1, HID], BF16),
        ("w3ta", [128, NKC, BITS], BF16),
        ("w3b", [1, BITS], BF16),
        ("wi1bt", [BITS, HID], BF16),
        ("posb", [128, NKC * BITS], F32),
        *[(f"w2dr{p}", [128, 2, HID], FP8) for p in range(2)],
        ("bi2c", [128, NKC], F32),
        *[(f"w3dr{p}", [128, 2, BITS], FP8) for p in range(2)],
        ("bi3c2", [128, 1], F32),
        ("zsel2", [128, 191], BF16),
        ("ident", [128, 128], BF16),
    ]
    if affine:
        specs += [
            ("g1bc", [128, HID], F32),
            ("be1bc", [128, HID], F32),
            ("g2bc", [128, HID], F32),
            ("be2bc", [128, HID], F32),
        ]
    return specs


@functools.lru_cache(maxsize=2)
def _get_nc(affine: bool = False):
    nc = bacc.Bacc("TRN2", target_bir_lowering=False, debug=False, num_devices=NCORES)
    io = {}
    for name, shape, dt in _input_specs(affine):
        io[name] = nc.dram_tensor(name, shape, dt, kind="ExternalInput").ap()
    io["out_t"] = nc.dram_tensor("out_t", [BITS, BC], F32, kind="ExternalOutput").ap()
    with tile.TileContext(nc) as tc:
        _build_kernel(tc, io, affine)
    nc.compile()
    return nc


def _host_prep(inputs, affine):
    """Shared (replicated) weight-derived tensors."""
    import ml_dtypes

    f = lambda x: np.ascontiguousarray(np.asarray(x, dtype=np.float32))
    bf = lambda x: np.ascontiguousarray(x).astype(ml_dtypes.bfloat16)
    f8 = lambda x: np.ascontiguousarray(x).astype(ml_dtypes.float8_e4m3)
    W1, b1 = f(inputs["W1"]), f(inputs["b1"])
    W2, b2 = f(inputs["W2"]), f(inputs["b2"])
    W3, b3 = f(inputs["W3"]), f(inputs["b3"])
    Wi1, bi1 = f(inputs["Wi1"]), f(inputs["bi1"])
    Wi2, bi2 = f(inputs["Wi2"]), f(inputs["bi2"])
    Wi3, bi3 = f(inputs["Wi3"]), f(inputs["bi3"])
    g1, be1 = f(inputs["g1"]), f(inputs["be1"])
    g2, be2 = f(inputs["g2"]), f(inputs["be2"])

    s = {}
    s["onesr"] = bf(np.ones((1, BC), np.float32))
    s["w1t"] = bf(np.vstack([W1.T, b1[None, :]]))
    s["w2ta"] = bf(W2.T.reshape(NKC, 128, HID).transpose(1, 0, 2))
    s["w2b"] = bf(b2[None, :])
    s["w3ta"] = bf(W3.T.reshape(NKC, 128, BITS).transpose(1, 0, 2))
    s["w3b"] = bf(b3[None, :])
    if affine:
        s["g1bc"] = np.broadcast_to(g1[None, :], (128, HID)).copy()
        s["be1bc"] = np.broadcast_to(be1[None, :], (128, HID)).copy()
        s["g2bc"] = np.broadcast_to(g2[None, :], (128, HID)).copy()
        s["be2bc"] = np.broadcast_to(be2[None, :], (128, HID)).copy()
    s["wi1bt"] = bf(Wi1[:, BITS:].T * S1)
    posb_full = (Wi1[:, :BITS] + bi1[:, None]) * S1            # [512, 64]
    s["posb"] = np.ascontiguousarray(
        posb_full.reshape(NKC, 128, BITS).transpose(1, 0, 2).reshape(128, NKC * BITS)
    )
    w2s = Wi2.T * S2                                           # [h, k]
    for p in range(2):
        s[f"w2dr{p}"] = f8(
            np.stack(
                [w2s[(2 * p) * 128 : (2 * p + 1) * 128],
                 w2s[(2 * p + 1) * 128 : (2 * p + 2) * 128]],
                axis=1,
            )
        )
    s["bi2c"] = np.ascontiguousarray((bi2 * S3).reshape(NKC, 128).T)
    w3s = Wi3.T * S4                                           # [h, j]
    for p in range(2):
        s[f"w3dr{p}"] = f8(
            np.stack(
                [w3s[(2 * p) * 128 : (2 * p + 1) * 128],
                 w3s[(2 * p + 1) * 128 : (2 * p + 2) * 128]],
                axis=1,
            )
        )
    s["bi3c2"] = np.concatenate([bi3, bi3])[:, None].copy()
    zsel2 = np.zeros((128, 191), np.float32)
    zsel2[:BITS, 127] = 1.0
    zsel2[BITS:, 128] = 1.0
    s["zsel2"] = bf(zsel2)
    s["ident"] = bf(np.eye(128, dtype=np.float32))
    return s


def _needs_affine(inputs):
    return not (
        np.all(np.asarray(inputs["g1"]) == 1.0)
        and np.all(np.asarray(inputs["g2"]) == 1.0)
        and np.all(np.asarray(inputs["be1"]) == 0.0)
        and np.all(np.asarray(inputs["be2"]) == 0.0)
        and np.all(np.asarray(inputs["b2"]) == 0.0)
        and np.all(np.asarray(inputs["b3"]) == 0.0)
    )


def _make_in_maps(inputs, affine=None):
    import ml_dtypes

    if affine is None:
        affine = _needs_affine(inputs)
    shared = _host_prep(inputs, affine)
    a_bits = np.asarray(inputs["a_bits"], dtype=np.float32)
    shift_bits = np.asarray(inputs["shift_bits"], dtype=np.float32)
    in_maps = []
    for c in range(NCORES):
        rows = slice(c * BC, (c + 1) * BC)
        m = dict(shared)
        m["sbT"] = np.ascontiguousarray(shift_bits[rows].T).astype(ml_dtypes.bfloat16)
        aT = np.ascontiguousarray(a_bits[rows].T)
        m["aTb2"] = np.vstack([aT, aT]).astype(ml_dtypes.bfloat16)
        in_maps.append(m)
    return in_maps


def assemble_output(results):
    out = np.empty((B, BITS), dtype=np.float32)
    for c in range(NCORES):
        sd = results[c]["out_t"]          # [BITS, BC] = d/s already divided
        out[c * BC : (c + 1) * BC] = sd.T
    return out


def run_on_cores(inputs, trace=False):
    """Returns (full_output [4096, 64] f32, BassKernelResults | None)."""
    affine = _needs_affine(inputs)
    nc = _get_nc(affine)
    in_maps = _make_in_maps(inputs, affine)
    if trace:
        res = bass_utils.run_bass_kernel_spmd(
            nc, in_maps, list(range(NCORES)), trace=True
        )
        return assemble_output(res.results), res
    from concourse import bass2jax

    results = bass2jax.run_bass_via_pjrt(nc, in_maps, n_cores=NCORES)
    return assemble_output(results), None


def kernel(**inputs) -> np.ndarray:
    out, _ = run_on_cores(inputs, trace=False)
    return out


# revision 18
# speedup vs baseline: 1.0347x; 1.0347x over previous
"""Bass/Tile TRN2 kernel for nn_DecomposedRotateNet (dense_mlp).

Contract: kernel(**inputs) takes FULL unsharded numpy inputs (as produced by
setup_inputs()) and returns the FULL [4096, 64] float32 output.

Strategy: pure data parallel over 8 NeuronCores -- batch 4096 -> 512 rows/core,
small MLP weights replicated; all layout prep on host.

v2: the dominant index-net matmuls run in fp8 e4m3 with DoubleRow perf mode
(2 fp8 weights/PE cell -> K=256 per matmul), with power-of-two scales folded
into the weights/biases host-side so intermediate activations sit in fp8's
good range.  The per-position softmax column sums are accumulated directly
into two PSUM banks via one-hot-column stationary operands (out row t <-
colsum of position t), so the normalize is two [64,512] ops at the very end
instead of per-position work.  Elementwise work is spread across DVE, ACT
and GPSIMD.
"""

import os
import sys
import functools

import numpy as np

for _p in ("/opt/trn_rl_repo",):
    if _p not in sys.path and os.path.isdir(_p):
        sys.path.insert(0, _p)

import concourse.bacc as bacc
import concourse.bass as bass
import concourse.mybir as mybir
import concourse.tile as tile
from concourse import bass_utils
from concourse._compat import with_exitstack
from contextlib import ExitStack

B, BITS, HID = 4096, 64, 512
NCORES = 8
BC = B // NCORES          # 512 batch rows per core
NBT = BC // 128           # 4 batch tiles of 128 (phase 1)
NKC = HID // 128          # 4 chunks of the hidden dim
EPS = 1e-5

# fp8 scale folding for the index-net (phase 2) matmuls
S1 = 64.0                 # h1 activation scale
S2 = 32.0                 # Wi2 weight scale
S3 = S1 * S2              # h2 activation scale (folded: no rescale op needed)
S4 = 64.0                 # Wi3 weight scale

F32 = mybir.dt.float32
BF16 = mybir.dt.bfloat16
FP8 = mybir.dt.float8e4

DR = mybir.MatmulPerfMode.DoubleRow


@with_exitstack
def _build_kernel(ctx: ExitStack, tc: "tile.TileContext", io: dict, affine: bool):
    nc = tc.nc
    AF = mybir.ActivationFunctionType
    ALU = mybir.AluOpType

    persist = ctx.enter_context(tc.tile_pool(name="persist", bufs=1))

    def load(name, shape, dt):
        t = persist.tile(shape, dt, name=f"sb_{name}", tag=f"sb_{name}")
        nc.sync.dma_start(t[:], io[name][:])
        return t

    def load_on(eng, name, shape, dt):
        t = persist.tile(shape, dt, name=f"sb_{name}", tag=f"sb_{name}")
        eng.dma_start(t[:], io[name][:])
        return t

    # ---- persistent SBUF tensors --------------------------------------
    # phase-1-critical tensors go first (and on separate queues) so the
    # first matmuls don't wait behind the phase-2 weight loads.
    w1t = load("w1t", [BITS + 1, HID], BF16)      # [W1.T ; b1]
    ident_early = None
    w2ta = load_on(nc.gpsimd, "w2ta", [128, NKC, HID], BF16)
    w2b = load_on(nc.gpsimd, "w2b", [1, HID], BF16)
    w3ta = load_on(nc.gpsimd, "w3ta", [128, NKC, BITS], BF16)
    w3b = load_on(nc.gpsimd, "w3b", [1, BITS], BF16)
    w2t = [w2ta[:, i, :] for i in range(NKC)]
    w3t = [w3ta[:, i, :] for i in range(NKC)]
    aTb2 = load_on(nc.scalar, "aTb2", [128, BC], BF16)  # a_bits .T, duplicated
    if affine:
        g1bc = load("g1bc", [128, HID], F32)
        be1bc = load("be1bc", [128, HID], F32)
        g2bc = load("g2bc", [128, HID], F32)
        be2bc = load("be2bc", [128, HID], F32)
    wi1bt = load_on(nc.gpsimd, "wi1bt", [BITS, HID], BF16)  # Wi1[:, 64:].T * S1
    posb = load_on(nc.scalar, "posb", [128, NKC * BITS], F32)
    w2dr = [load_on(nc.scalar, f"w2dr{p}", [128, 2, HID], FP8) for p in range(2)]
    bi2c = load_on(nc.scalar, "bi2c", [128, NKC], F32)
    w3dr = [load_on(nc.scalar, f"w3dr{p}", [128, 2, BITS], FP8) for p in range(2)]
    bi3c2 = load_on(nc.scalar, "bi3c2", [128, 1], F32)      # bi3 duplicated
    zsel2 = load_on(nc.scalar, "zsel2", [128, 191], BF16)   # pair one-hot columns

    # x0a = [shift_bits_T shard ; ones row] for the z1 matmul (bias fold)
    x0a = persist.tile([BITS + 1, BC], BF16, name="x0a", tag="x0a")
    nc.sync.dma_start(x0a[0:BITS, :], io["sbT"][:])
    nc.sync.dma_start(x0a[BITS : BITS + 1, :], io["onesr"][:])
    ones1r = persist.tile([1, BC], BF16, name="ones1r", tag="ones1r")
    nc.sync.dma_start(ones1r[:], io["onesr"][:])
    ident = load("ident", [128, 128], BF16)

    epsc = persist.tile([128, 1], F32, name="epsc", tag="epsc")
    nc.vector.memset(epsc[:], EPS)
    # prefetch activation-function tables while input DMAs run, so the
    # first LN/softmax ops don't serialize behind ACT_TABLE_LOADs
    warmup = persist.tile([1, 1], F32, name="warmup", tag="warmup")
    for fn in (AF.Sqrt, AF.Relu, AF.Exp, AF.Identity):
        nc.scalar.activation(warmup[:], epsc[0:1, :], fn)

    ssT = persist.tile([BITS, BC], BF16, name="ssT", tag="ssT")      # shift_soft.T
    shiftT = persist.tile([128, NKC, BC], BF16, name="shiftT", tag="shiftT")

    # =================== phase 1: shift decoder =======================
    # Stage-major across the 4 batch tiles so the serial LN dependency
    # chains of different tiles overlap on DVE/ACT while PE runs matmuls.
    with (
        tc.tile_pool(name="p1s", bufs=4) as p1s,
        tc.tile_pool(name="p1p", bufs=2, space="PSUM") as p1p,
    ):
        def layernorm_relu(z_psum, g, be, out_tag, bt):
            """z [128, 512] PSUM -> relu(LN(z)*g+be) [128, 512] SBUF bf16."""
            stats = p1s.tile([128, 6], F32, tag=f"stats{bt}", name="stats")
            nc.vector.bn_stats(stats[:], z_psum[:])
            mv = p1s.tile([128, 2], F32, tag=f"mv{bt}", name="mv")
            nc.vector.bn_aggr(mv[:], stats[:])
            std = p1s.tile([128, 1], F32, tag=f"std{bt}", name="std")
            nc.scalar.activation(std[:], mv[:, 1:2], AF.Sqrt, bias=epsc[:])
            rinv = p1s.tile([128, 1], F32, tag=f"rinv{bt}", name="rinv")
            nc.vector.reciprocal(rinv[:], std[:])
            nmr = p1s.tile([128, 1], F32, tag=f"nmr{bt}", name="nmr")
            # nmr = (mean * -1) * rinv
            nc.vector.scalar_tensor_tensor(
                nmr[:], mv[:, 0:1], -1.0, rinv[:], op0=ALU.mult, op1=ALU.mult
            )
            if not affine:
                # g==1, be==0: relu(LN(z)) in one ACT op
                a = p1s.tile([128, HID], BF16, tag=out_tag, name="a")
                nc.scalar.activation(a[:], z_psum[:], AF.Relu, bias=nmr[:], scale=rinv[:])
                return a
            xn = p1s.tile([128, HID], F32, tag=f"xn{bt}", name="xn")
            nc.scalar.activation(xn[:], z_psum[:], AF.Identity, bias=nmr[:], scale=rinv[:])
            t1 = p1s.tile([128, HID], F32, tag=f"t1{bt}", name="t1")
            nc.vector.tensor_tensor(t1[:], xn[:], g[:], op=ALU.mult)
            t2 = p1s.tile([128, HID], F32, tag=f"t2{bt}", name="t2")
            nc.vector.tensor_tensor(t2[:], t1[:], be[:], op=ALU.add)
            a = p1s.tile([128, HID], BF16, tag=out_tag, name="a")
            nc.vector.tensor_scalar_max(a[:], t2[:], 0.0)
            return a

        def transpose128(src, cols, out_tag):
            """src [128, cols*128] SBUF bf16 -> list of [128,128] transposed."""
            outs = []
            for h in range(cols):
                tp = p1p.tile([128, 128], BF16, tag="tp", bufs=2, name="tp")
                nc.tensor.transpose(tp[:], src[:, h * 128 : (h + 1) * 128], ident[:])
                sb = p1s.tile([128, 128], BF16, tag=f"{out_tag}{h}", name="sb")
                nc.vector.tensor_copy(sb[:], tp[:])
                outs.append(sb)
            return outs

        g1 = g1bc if affine else None
        be1 = be1bc if affine else None
        g2 = g2bc if affine else None
        be2 = be2bc if affine else None
        bss = [slice(bt * 128, (bt + 1) * 128) for bt in range(NBT)]
        z1s, a1s, a1Ts, z2s, a2s, a2Ts, z3s = {}, {}, {}, {}, {}, {}, {}
        for bt in range(NBT):
            z1s[bt] = p1p.tile([128, HID], F32, tag=f"zz{bt}", name="z1", bufs=1)
            nc.tensor.matmul(z1s[bt][:], x0a[:, bss[bt]], w1t[:], start=True, stop=True)
        for bt in range(NBT):
            a1s[bt] = layernorm_relu(z1s[bt], g1, be1, f"a1_{bt}", bt)
        for bt in range(NBT):
            a1Ts[bt] = transpose128(a1s[bt], NKC, f"a1T_{bt}_")
            z2s[bt] = p1p.tile([128, HID], F32, tag=f"zz{bt}", name="z2", bufs=1)
            for h in range(NKC):
                nc.tensor.matmul(z2s[bt][:], a1Ts[bt][h][:], w2t[h], start=(h == 0), stop=(not affine and h == NKC - 1))
            if affine:
                nc.tensor.matmul(z2s[bt][:], ones1r[:, bss[bt]], w2b[:], start=False, stop=True)
        for bt in range(NBT):
            a2s[bt] = layernorm_relu(z2s[bt], g2, be2, f"a2_{bt}", bt)
        for bt in range(NBT):
            a2Ts[bt] = transpose128(a2s[bt], NKC, f"a2T_{bt}_")
            z3s[bt] = p1p.tile([128, BITS], F32, tag=f"z3_{bt%2}", name="z3", bufs=1)
            for h in range(NKC):
                nc.tensor.matmul(z3s[bt][:], a2Ts[bt][h][:], w3t[h], start=(h == 0), stop=(not affine and h == NKC - 1))
            if affine:
                nc.tensor.matmul(z3s[bt][:], ones1r[:, bss[bt]], w3b[:], start=False, stop=True)
        for bt in range(NBT):
            # softmax over free dim (64)
            z3 = z3s[bt]
            mx = p1s.tile([128, 1], F32, tag=f"mx{bt}", name="mx")
            nc.vector.reduce_max(mx[:], z3[:], axis=mybir.AxisListType.X)
            nmx = p1s.tile([128, 1], F32, tag=f"nmx{bt}", name="nmx")
            nc.vector.tensor_scalar_mul(nmx[:], mx[:], -1.0)
            es = p1s.tile([128, BITS], F32, tag=f"es{bt}", name="es")
            ssum = p1s.tile([128, 1], F32, tag=f"ssum{bt}", name="ssum")
            nc.scalar.activation(es[:], z3[:], AF.Exp, bias=nmx[:], accum_out=ssum[:])
            rs = p1s.tile([128, 1], F32, tag=f"rs{bt}", name="rs")
            nc.vector.reciprocal(rs[:], ssum[:])
            ss = p1s.tile([128, BITS], BF16, tag=f"ss{bt}", name="ss")
            nc.vector.tensor_scalar_mul(ss[:], es[:], rs[:])
            tps = p1p.tile([BITS, 128], BF16, tag="tp", bufs=2, name="tps")
            nc.tensor.transpose(tps[:], ss[:], ident[:])
            nc.vector.tensor_copy(ssT[:, bss[bt]], tps[:])

        # ---- phase 1.5: shiftT = (Wi1[:,64:]*S1).T-chunks @ shift_soft.T
        sps = {}
        for fc in range(NKC):
            sps[fc] = p1p.tile([128, BC], F32, tag=f"zz{fc}", name="sp", bufs=1)
            nc.tensor.matmul(
                sps[fc][:], wi1bt[:, fc * 128 : (fc + 1) * 128], ssT[:], start=True, stop=True
            )
        for fc in range(NKC):
            nc.vector.tensor_copy(shiftT[:, fc, :], sps[fc][:])

    # =================== phase 2: index net (fp8 DoubleRow) ============
    # 64 output positions t, processed in 16 blocks of 4 (BLK) so each
    # MM1 weight tile is loaded once per block (LDWEIGHTS amortized 4x).
    # Positions are paired for the epilogue: MM2 writes logits for even t
    # to PSUM partitions 0-63 and odd t to 64-127, so exp / a-weighting /
    # column sums run on full [128, 512] tiles (one op per 2 positions).
    # Column sums accumulate into sAll/dAll PSUM banks via one-hot-column
    # stationary operands; the final normalize is 2 ops + 1 DMA.
    NP = BITS  # 64 output positions
    BLK = 4
    NBLK = NP // BLK
    with (
        tc.tile_pool(name="p2s", bufs=2) as p2s,
        tc.tile_pool(name="p2z", bufs=1, space="PSUM") as p2z,
        tc.tile_pool(name="p2lg", bufs=1, space="PSUM") as p2lg,
        tc.tile_pool(name="p2acc", bufs=1, space="PSUM") as p2acc,
    ):
        sAll = p2acc.tile([BITS, BC], F32, tag="sAll")
        dAll = p2acc.tile([BITS, BC], F32, tag="dAll")

        st = {}

        def h1_ops(g, slot):
            """h1(t) = relu(shiftT + posb[:, t-col]) -> fp8, 4 chunks."""
            t = g * BLK + slot
            h1 = st[(g, "h1", slot)]
            for fc in range(NKC):
                b = posb[:, fc * BITS + t : fc * BITS + t + 1]
                dst = h1[:, fc, :]
                src = shiftT[:, fc, :]
                if fc % 2 == 0:
                    nc.vector.tensor_scalar(
                        dst, src, b, 0.0, op0=ALU.add, op1=ALU.max
                    )
                else:
                    nc.scalar.activation(dst, src, AF.Relu, bias=b)

        def alloc_h1_block(g):
            for slot in range(BLK):
                st[(g, "h1", slot)] = p2s.tile(
                    [128, NKC, BC], FP8, tag=f"h1_{slot}", name=f"h1_{slot}"
                )

        def expcs_pair(g, pair):
            """exp + a-weight + column sums for pair (2 positions)."""
            lgp = st[(g, "lg", pair)]
            e2 = p2s.tile([128, BC], BF16, tag="e2", name="e2")
            nc.scalar.activation(
                e2[:], lgp[:], AF.Exp, bias=bi3c2[:], scale=1.0 / (S3 * S4)
            )
            tmp2 = p2s.tile([128, BC], BF16, tag="tmp2", name="tmp2")
            nc.vector.tensor_tensor(tmp2[:], e2[:], aTb2[:], op=ALU.mult)
            pi = g * 2 + pair                     # global pair index [0, 32)
            sel = zsel2[:, 127 - 2 * pi : 191 - 2 * pi]
            nc.tensor.matmul(
                sAll[:], sel, e2[:],
                start=(pi == 0), stop=(pi == NP // 2 - 1), skip_group_check=True,
            )
            nc.tensor.matmul(
                dAll[:], sel, tmp2[:],
                start=(pi == 0), stop=(pi == NP // 2 - 1), skip_group_check=True,
            )
            del st[(g, "lg", pair)]

        def mm1_block(g):
            """MM1 for 4 positions, weight-stationary over (p, kc); h2 ops
            drain each kc round; prev block's epilogue and next block's h1
            are interleaved to keep all engines fed."""
            h2b = [
                p2s.tile([128, NKC, BC], FP8, tag=f"h2_{s}", name=f"h2_{s}")
                for s in range(BLK)
            ]
            for kc in range(NKC):
                z = [
                    p2z.tile([128, BC], F32, tag=f"z{s}", name=f"z{s}")
                    for s in range(BLK)
                ]
                for p in range(2):
                    w = w2dr[p][:, :, kc * 128 : (kc + 1) * 128]
                    for s in range(BLK):
                        nc.tensor.matmul(
                            z[s][:],
                            w,
                            st[(g, "h1", s)][:, 2 * p : 2 * p + 2, :],
                            start=(p == 0),
                            stop=(p == 1),
                            perf_mode=DR,
                            skip_group_check=True,
                        )
                # per-slot h2 so each z bank frees as early as possible
                b = bi2c[:, kc : kc + 1]
                for s in range(BLK):
                    dst = h2b[s][:, kc, :]
                    if (kc + s) % 2 == 0:
                        nc.vector.tensor_scalar(
                            dst, z[s][:], b, 0.0, op0=ALU.add, op1=ALU.max
                        )
                    else:
                        nc.scalar.activation(dst, z[s][:], AF.Relu, bias=b)
                # interleave: prev-block mm2 + epilogue pairs, next-block h1
                # (h1 production front-loaded into the PE-heavy kc0 round)
                if kc == 0 and g >= 1:
                    mm2_block(g - 1)
                elif kc == 1 and g >= 1:
                    expcs_pair(g - 1, 0)
                elif kc == 2 and g >= 1:
                    expcs_pair(g - 1, 1)
                if g + 1 < NBLK:
                    if kc == 0:
                        h1_ops(g + 1, 0)
                        h1_ops(g + 1, 1)
                    elif kc == 1:
                        h1_ops(g + 1, 2)
                    elif kc == 2:
                        h1_ops(g + 1, 3)
            for s in range(BLK):
                st[(g, "h2", s)] = h2b[s]
                del st[(g, "h1", s)]

        def mm2_block(g):
            for pair in range(2):
                lgp = p2lg.tile([128, BC], F32, tag=f"lgp{pair}", name=f"lgp{pair}")
                for par in range(2):
                    s = 2 * pair + par
                    h2 = st[(g, "h2", s)]
                    out = lgp[64 * par : 64 * (par + 1), :]
                    if par == 0:
                        # DoubleRow requires dst partition 0
                        for p in range(2):
                            nc.tensor.matmul(
                                out,
                                w3dr[p][:, :, :],
                                h2[:, 2 * p : 2 * p + 2, :],
                                start=(p == 0),
                                stop=(p == 1),
                                perf_mode=DR,
                                skip_group_check=True,
                            )
                    else:
                        # odd slot -> partitions 64-127: plain fp8 matmuls
                        for fc in range(NKC):
                            nc.tensor.matmul(
                                out,
                                w3dr[fc // 2][:, fc % 2, :],
                                h2[:, fc, :],
                                start=(fc == 0),
                                stop=(fc == NKC - 1),
                                skip_group_check=True,
                            )
                st[(g, "lg", pair)] = lgp
            for s in range(BLK):
                del st[(g, "h2", s)]

        alloc_h1_block(0)
        for slot in range(BLK):
            h1_ops(0, slot)
        for g in range(NBLK):
            if g + 1 < NBLK:
                alloc_h1_block(g + 1)
            mm1_block(g)
        mm2_block(NBLK - 1)
        expcs_pair(NBLK - 1, 0)
        expcs_pair(NBLK - 1, 1)

        # final normalize: out[t, b] = dAll/sAll
        r = p2s.tile([BITS, BC], F32, tag="r")
        nc.vector.reciprocal(r[:], sAll[:])
        outT = p2s.tile([BITS, BC], F32, tag="outT")
        nc.vector.tensor_tensor(outT[:], dAll[:], r[:], op=ALU.mult)
        nc.sync.dma_start(io["out_t"][:], outT[:])


def _input_specs(affine: bool):
    specs = [
        ("sbT", [BITS, BC], BF16),
        ("aTb2", [128, BC], BF16),
        ("onesr", [1, BC], BF16),
        ("w1t", [BITS + 1, HID], BF16),
        ("w2ta", [128, NKC, HID], BF16),
        ("w2b", [1, HID], BF16),
        ("w3ta", [128, NKC, BITS], BF16),
        ("w3b", [1, BITS], BF16),
        ("wi1bt", [BITS, HID], BF16),
        ("posb", [128, NKC * BITS], F32),
        *[(f"w2dr{p}", [128, 2, HID], FP8) for p in range(2)],
        ("bi2c", [128, NKC], F32),
        *[(f"w3dr{p}", [128, 2, BITS], FP8) for p in range(2)],
        ("bi3c2", [128, 1], F32),
        ("zsel2", [128, 191], BF16),
        ("ident", [128, 128], BF16),
    ]
    if affine:
        specs += [
            ("g1bc", [128, HID], F32),
            ("be1bc", [128, HID], F32),
            ("g2bc", [128, HID], F32),
            ("be2bc", [128, HID], F32),
        ]
    return specs


@functools.lru_cache(maxsize=2)
def _get_nc(affine: bool = False):
    nc = bacc.Bacc("TRN2", target_bir_lowering=False, debug=False, num_devices=NCORES)
    io = {}
    for name, shape, dt in _input_specs(affine):
        io[name] = nc.dram_tensor(name, shape, dt, kind="ExternalInput").ap()
    io["out_t"] = nc.dram_tensor("out_t", [BITS, BC], F32, kind="ExternalOutput").ap()
    with tile.TileContext(nc) as tc:
        _build_kernel(tc, io, affine)
    nc.compile()
    return nc


def _host_prep(inputs, affine):
    """Shared (replicated) weight-derived tensors."""
    import ml_dtypes

    f = lambda x: np.ascontiguousarray(np.asarray(x, dtype=np.float32))
    bf = lambda x: np.ascontiguousarray(x).astype(ml_dtypes.bfloat16)
    f8 = lambda x: np.ascontiguousarray(x).astype(ml_dtypes.float8_e4m3)
    W1, b1 = f(inputs["W1"]), f(inputs["b1"])
    W2, b2 = f(inputs["W2"]), f(inputs["b2"])
    W3, b3 = f(inputs["W3"]), f(inputs["b3"])
    Wi1, bi1 = f(inputs["Wi1"]), f(inputs["bi1"])
    Wi2, bi2 = f(inputs["Wi2"]), f(inputs["bi2"])
    Wi3, bi3 = f(inputs["Wi3"]), f(inputs["bi3"])
    g1, be1 = f(inputs["g1"]), f(inputs["be1"])
    g2, be2 = f(inputs["g2"]), f(inputs["be2"])

    s = {}
    s["onesr"] = bf(np.ones((1, BC), np.float32))
    s["w1t"] = bf(np.vstack([W1.T, b1[None, :]]))
    s["w2ta"] = bf(W2.T.reshape(NKC, 128, HID).transpose(1, 0, 2))
    s["w2b"] = bf(b2[None, :])
    s["w3ta"] = bf(W3.T.reshape(NKC, 128, BITS).transpose(1, 0, 2))
    s["w3b"] = bf(b3[None, :])
    if affine:
        s["g1bc"] = np.broadcast_to(g1[None, :], (128, HID)).copy()
        s["be1bc"] = np.broadcast_to(be1[None, :], (128, HID)).copy()
        s["g2bc"] = np.broadcast_to(g2[None, :], (128, HID)).copy()
        s["be2bc"] = np.broadcast_to(be2[None, :], (128, HID)).copy()
    s["wi1bt"] = bf(Wi1[:, BITS:].T * S1)
    posb_full = (Wi1[:, :BITS] + bi1[:, None]) * S1            # [512, 64]
    s["posb"] = np.ascontiguousarray(
        posb_full.reshape(NKC, 128, BITS).transpose(1, 0, 2).reshape(128, NKC * BITS)
    )
    w2s = Wi2.T * S2                                           # [h, k]
    for p in range(2):
        s[f"w2dr{p}"] = f8(
            np.stack(
                [w2s[(2 * p) * 128 : (2 * p + 1) * 128],
                 w2s[(2 * p + 1) * 128 : (2 * p + 2) * 128]],
                axis=1,
            )
        )
    s["bi2c"] = np.ascontiguousarray((bi2 * S3).reshape(NKC, 128).T)
    w3s = Wi3.T * S4                                           # [h, j]
    for p in range(2):
        s[f"w3dr{p}"] = f8(
            np.stack(
                [w3s[(2 * p) * 128 : (2 * p + 1) * 128],
                 w3s[(2 * p + 1) * 128 : (2 * p + 2) * 128]],
                axis=1,
            )
        )
    s["bi3c2"] = np.concatenate([bi3, bi3])[:, None].copy()
    zsel2 = np.zeros((128, 191), np.float32)
    zsel2[:BITS, 127] = 1.0
    zsel2[BITS:, 128] = 1.0
    s["zsel2"] = bf(zsel2)
    s["ident"] = bf(np.eye(128, dtype=np.float32))
    return s


def _needs_affine(inputs):
    return not (
        np.all(np.asarray(inputs["g1"]) == 1.0)
        and np.all(np.asarray(inputs["g2"]) == 1.0)
        and np.all(np.asarray(inputs["be1"]) == 0.0)
        and np.all(np.asarray(inputs["be2"]) == 0.0)
        and np.all(np.asarray(inputs["b2"]) == 0.0)
        and np.all(np.asarray(inputs["b3"]) == 0.0)
    )


def _make_in_maps(inputs, affine=None):
    import ml_dtypes

    if affine is None:
        affine = _needs_affine(inputs)
    shared = _host_prep(inputs, affine)
    a_bits = np.asarray(inputs["a_bits"], dtype=np.float32)
    shift_bits = np.asarray(inputs["shift_bits"], dtype=np.float32)
    in_maps = []
    for c in range(NCORES):
        rows = slice(c * BC, (c + 1) * BC)
        m = dict(shared)
        m["sbT"] = np.ascontiguousarray(shift_bits[rows].T).astype(ml_dtypes.bfloat16)
        aT = np.ascontiguousarray(a_bits[rows].T)
        m["aTb2"] = np.vstack([aT, aT]).astype(ml_dtypes.bfloat16)
        in_maps.append(m)
    return in_maps


def assemble_output(results):
    out = np.empty((B, BITS), dtype=np.float32)
    for c in range(NCORES):
        sd = results[c]["out_t"]          # [BITS, BC] = d/s already divided
        out[c * BC : (c + 1) * BC] = sd.T
    return out


def run_on_cores(inputs, trace=False):
    """Returns (full_output [4096, 64] f32, BassKernelResults | None)."""
    affine = _needs_affine(inputs)
    nc = _get_nc(affine)
    in_maps = _make_in_maps(inputs, affine)
    if trace:
        res = bass_utils.run_bass_kernel_spmd(
            nc, in_maps, list(range(NCORES)), trace=True
        )
        return assemble_output(res.results), res
    from concourse import bass2jax

    results = bass2jax.run_bass_via_pjrt(nc, in_maps, n_cores=NCORES)
    return assemble_output(results), None


def kernel(**inputs) -> np.ndarray:
    out, _ = run_on_cores(inputs, trace=False)
    return out
